# revision 7
# baseline (speedup 1.0000x reference)
"""Trainium2 Bass kernel for the Context Encoder problem:

    ce  = c2e_weight[nodes]            # [N, 128] embedding gather
    h   = relu(ce @ w1.T + b1)         # [N, 128]
    out = relu(h @ w2.T + b2)          # [N, 128]

Strategy (8 NeuronCores, vocab-range sharding):
  200000 node ids over a 100000-row vocab saturate every vocab window,
  so transforming the table itself is less work than gathering per-node
  rows (and avoids the per-index DMA descriptor-generation cost that
  dominates any on-device gather).

  - The vocab is split into 8 fixed 12500-row ranges.  Core i streams
    its host-pre-transposed (d-major) bf16 table window [128, 12544]
    contiguously at full DMA bandwidth and computes
    T2 = relu(relu(win @ w1.T + b1) @ w2.T + b2) for every window row.
  - bf16 operands run the PE at 1 cycle/column (4x the fp32 rate) and
    halve both DMA directions; PSUM accumulation stays fp32 so the
    only precision loss is the bf16 quantization of table/weights/out
    (~4e-3 rel vs the 2e-2 gate).
  - All input-chunk DMAs are triggered up front on the Sync queue so
    the input stream never stalls behind compute; outputs accumulate
    in an SBUF staging tile and leave as four large DMAs issued from
    the otherwise-idle GpSimd queue.  Each dma_start costs ~600ns of
    sequencer descriptor generation, so DMA count is kept minimal.
  - Super-tiles span two PSUM banks (1024 columns); each layer is two
    512-column matmuls back-to-back, then a single 1024-wide bias+relu
    (layer 1 on ACT, layer 2 on DVE).  Tiles are software-pipelined one
    deep so mm2 of tile k-1 issues after mm1 of tile k and the PE does
    not stall on the ACT engine.
  - Results stay feature-major; the host transposes each window and maps
    node positions to rows (out = T2[nodes]) as the unshard step.
"""

import os
import sys

for _p in ("/opt/trn_rl_repo",):
    if _p not in sys.path:
        sys.path.insert(0, _p)

import ml_dtypes
import numpy as np

import concourse.bass as bass
import concourse.mybir as mybir
import concourse.tile as tile
from concourse import bacc
from concourse.bass_utils import run_bass_kernel_spmd
from concourse.tile import TileContext

P = 128
D = 128
N_CORES = 8
VOCAB = 100000
RANGE = VOCAB // N_CORES   # 12500 vocab rows owned per core
BLOCKS = 98                # 12544 rows processed per core (128*98)
ROWS = BLOCKS * P
BF16 = ml_dtypes.bfloat16

CHUNKS = [2, 24, 24, 24, 24]   # input DMA chunk sizes (blocks)
TILE_BLOCKS = 8                # blocks per compute super-tile
MMW = 512                      # single-matmul width (one PSUM fp32 bank)
LOOKAHEAD = 2                  # super-tiles in flight between mm1 and mm2


def build_nc():
    f32 = mybir.dt.float32
    bf16 = mybir.dt.bfloat16
    nc = bacc.Bacc("TRN2", target_bir_lowering=False, debug=False,
                   num_devices=N_CORES)

    tsl_t = nc.dram_tensor("tslice", [P, ROWS], bf16,
                           kind="ExternalInput").ap()
    wpack_t = nc.dram_tensor("wpack", [P, 2 * D], bf16,
                             kind="ExternalInput").ap()
    bpack_t = nc.dram_tensor("bpack", [P, 2], f32,
                             kind="ExternalInput").ap()
    out_t = nc.dram_tensor("out", [P, ROWS], bf16,
                           kind="ExternalOutput").ap()

    fw_max = TILE_BLOCKS * P  # 1024

    # super-tiles: one small starter tile, then 1024-column tiles
    tiles = []
    s0 = 0
    first = CHUNKS[0] * P
    tiles.append((s0, first))
    s0 += first
    while s0 < ROWS:
        fw = min(fw_max, ROWS - s0)
        tiles.append((s0, fw))
        s0 += fw

    # output groups (tile counts): big batches early for few descriptors,
    # a small final group so the last DMA after the last relu2 is short
    group_sizes = [4, 3, 3, 2, 1]
    assert sum(group_sizes) == len(tiles)
    group_ends = []
    i = 0
    for g in group_sizes:
        i += g
        group_ends.append(tiles[i - 1][0] + tiles[i - 1][1])

    with TileContext(nc) as tc:
        with (
            tc.tile_pool(name="const", bufs=1) as cpool,
            tc.tile_pool(name="io", bufs=1) as iopool,
            tc.tile_pool(name="work", bufs=3) as wpool,
            tc.tile_pool(name="psum", bufs=2, space="PSUM") as ppool,
        ):
            wpack_sb = cpool.tile([P, 2 * D], bf16, tag="wpack")
            nc.sync.dma_start(out=wpack_sb[:], in_=wpack_t[:])
            bpack_sb = cpool.tile([P, 2], f32, tag="bpack")
            nc.sync.dma_start(out=bpack_sb[:], in_=bpack_t[:])
            w1t_sb = wpack_sb[:, 0:D]
            w2t_sb = wpack_sb[:, D:2 * D]
            b1_sb = bpack_sb[:, 0:1]
            b2_sb = bpack_sb[:, 1:2]

            # one big input tile; all chunk DMAs trigger up front
            win = iopool.tile([P, ROWS], bf16, tag="win")
            r = 0
            for cb in CHUNKS:
                cw = cb * P
                nc.sync.dma_start(out=win[:, r:r + cw],
                                  in_=tsl_t[:, r:r + cw])
                r += cw

            # output staging tile; relu2 writes slices, GpSimd drains
            ostage = iopool.tile([P, ROWS], bf16, tag="ostage")

            def mm_layer(out_ps, lhsT, rhs_ap, fw):
                for j in range(0, fw, MMW):
                    w = min(MMW, fw - j)
                    nc.tensor.matmul(out=out_ps[:, j:j + w], lhsT=lhsT,
                                     rhs=rhs_ap[:, j:j + w],
                                     start=True, stop=True)

            def stage_a(s0, fw):
                """mm1 + relu1(ACT) for one super-tile."""
                h_ps = ppool.tile([P, fw_max], f32, tag="h")
                mm_layer(h_ps, w1t_sb, win[:, s0:s0 + fw], fw)
                hT_sb = wpool.tile([P, fw_max], bf16, tag="hT", bufs=4)
                nc.scalar.activation(hT_sb[:, :fw], h_ps[:, :fw],
                                     mybir.ActivationFunctionType.Relu,
                                     bias=b1_sb)
                return hT_sb

            def stage_b(hT_sb, s0, fw):
                """mm2 + relu2(DVE) into the staging tile."""
                o_ps = ppool.tile([P, fw_max], f32, tag="o")
                mm_layer(o_ps, w2t_sb, hT_sb[:, :fw], fw)
                nc.vector.tensor_scalar(
                    out=ostage[:, s0:s0 + fw], in0=o_ps[:, :fw],
                    scalar1=b2_sb, scalar2=0.0,
                    op0=mybir.AluOpType.add, op1=mybir.AluOpType.max)

            pend = []  # (hT_sb, s0, fw) of in-flight super-tiles
            g_lo = 0
            gi = 0

            def maybe_drain(done_cols):
                nonlocal g_lo, gi
                if gi < len(group_ends) and done_cols >= group_ends[gi]:
                    hi = group_ends[gi]
                    nc.gpsimd.dma_start(out=out_t[:, g_lo:hi],
                                        in_=ostage[:, g_lo:hi])
                    g_lo = hi
                    gi += 1

            for (s0, fw) in tiles:
                hT_sb = stage_a(s0, fw)
                pend.append((hT_sb, s0, fw))
                if len(pend) > LOOKAHEAD:
                    b = pend.pop(0)
                    stage_b(*b)
                    maybe_drain(b[1] + b[2])
            for b in pend:
                stage_b(*b)
                maybe_drain(b[1] + b[2])

    nc.compile()
    return nc


_CACHED_NC = None
LAST_RESULTS = None


def _get_nc():
    global _CACHED_NC
    if _CACHED_NC is None:
        _CACHED_NC = build_nc()
    return _CACHED_NC


def kernel(nodes, c2e_weight, w1, b1, w2, b2):
    nodes = np.asarray(nodes).astype(np.int64)
    c2e_weight = np.asarray(c2e_weight, dtype=np.float32)
    w1 = np.asarray(w1, dtype=np.float32)
    b1 = np.asarray(b1, dtype=np.float32)
    w2 = np.asarray(w2, dtype=np.float32)
    b2 = np.asarray(b2, dtype=np.float32)

    vocab = c2e_weight.shape[0]
    assert vocab == VOCAB, vocab

    tableT = c2e_weight.T  # [128, VOCAB], d-major view

    wpack = np.concatenate(
        [np.ascontiguousarray(w1.T), np.ascontiguousarray(w2.T)],
        axis=1).astype(BF16)
    bpack = np.stack([b1, b2], axis=1).astype(np.float32)
    bpack = np.ascontiguousarray(bpack)

    starts = []
    in_maps = []
    for i in range(N_CORES):
        start = min(i * RANGE, vocab - ROWS)
        starts.append(start)
        in_maps.append({
            "tslice": np.ascontiguousarray(
                tableT[:, start:start + ROWS]).astype(BF16),
            "wpack": wpack,
            "bpack": bpack,
        })

    nc = _get_nc()
    trace = os.environ.get("BASS_KERNEL_TRACE") == "1"
    if trace:
        try:  # tracing needs the NTFF hook; degrade silently without it
            import antenv.axon_hooks  # noqa: F401
        except ImportError:
            trace = False
    res = run_bass_kernel_spmd(nc, in_maps, core_ids=list(range(N_CORES)),
                               trace=trace)
    global LAST_RESULTS
    LAST_RESULTS = res

    # T2[v] = MLP(c2e_weight[v]) assembled from the 8 windows
    t2 = np.empty((vocab, D), dtype=np.float32)
    for i in range(N_CORES):
        dense = res.results[i]["out"]                    # [128, ROWS] bf16
        lo = i * RANGE
        hi = min((i + 1) * RANGE, vocab)
        t2[lo:hi] = dense[:, lo - starts[i]:hi - starts[i]].T

    return t2[nodes]


# revision 14
# speedup vs baseline: 1.0602x; 1.0602x over previous
"""Trainium2 Bass kernel for the Context Encoder problem:

    ce  = c2e_weight[nodes]            # [N, 128] embedding gather
    h   = relu(ce @ w1.T + b1)         # [N, 128]
    out = relu(h @ w2.T + b2)          # [N, 128]

Strategy (8 NeuronCores, vocab-range sharding):
  200000 node ids over a 100000-row vocab saturate every vocab window,
  so transforming the table itself is less work than gathering per-node
  rows (and avoids the per-index DMA descriptor-generation cost that
  dominates any on-device gather).

  - The vocab is split into 8 fixed 12500-row ranges.  Core i streams
    its host-pre-transposed (d-major) bf16 table window [128, 12544]
    contiguously at full DMA bandwidth and computes
    T2 = relu(relu(win @ w1.T + b1) @ w2.T + b2) for every window row.
  - bf16 operands run the PE at 1 cycle/column (4x the fp32 rate) and
    halve both DMA directions; PSUM accumulation stays fp32 so the
    only precision loss is the bf16 quantization of table/weights/out
    (~4e-3 rel vs the 2e-2 gate).
  - All input-chunk DMAs are triggered up front on the Sync queue so
    the input stream never stalls behind compute; outputs accumulate
    in an SBUF staging tile and leave as four large DMAs issued from
    the otherwise-idle GpSimd queue.  Each dma_start costs ~600ns of
    sequencer descriptor generation, so DMA count is kept minimal.
  - Super-tiles span two PSUM banks (1024 columns); each layer is two
    512-column matmuls back-to-back, then a single 1024-wide bias+relu
    (layer 1 on ACT, layer 2 on DVE).  Tiles are software-pipelined one
    deep so mm2 of tile k-1 issues after mm1 of tile k and the PE does
    not stall on the ACT engine.
  - Results stay feature-major; the host transposes each window and maps
    node positions to rows (out = T2[nodes]) as the unshard step.
"""

import os
import sys

for _p in ("/opt/trn_rl_repo",):
    if _p not in sys.path:
        sys.path.insert(0, _p)

import ml_dtypes
import numpy as np

import concourse.bass as bass
import concourse.mybir as mybir
import concourse.tile as tile
from concourse import bacc
from concourse.bass_utils import run_bass_kernel_spmd
from concourse.tile import TileContext
from concourse.vector_clock import ScopedClock


class FastExitTileContext(TileContext):
    """Single-use TileContext with a cheap exit sequence.

    The stock exit runs sync-drain -> full barrier -> sem clear -> full
    barrier, where each full barrier emits a per-engine InstDrain whose
    release chain serializes at ~1.3us per engine on TRN2 (~8us of
    measured kernel time).  The sync drain already waits on every
    completion semaphore (including all DMA queues), so for a program
    that ends right after the TileContext a sem-only barrier before the
    clear is sufficient, and nothing needs to run after the clear.
    """

    def _drain_and_barrier(self, tick_clock, wait_clock):
        drain_inst = self.nc.sync.drain()
        wait_clock.add_sem_waits(
            drain_inst.ins, ScopedClock({None: tick_clock.global_clock})
        )
        self.nc.all_engine_barrier(sem_only=True)
        popped = self.nc._tile_sem_poison_stack.pop()
        assert popped is self._sem_poison
        self.nc.clear_and_free_semaphores(list(self.sems.allocated().values()))

P = 128
D = 128
N_CORES = 8
VOCAB = 100000
RANGE = VOCAB // N_CORES   # 12500 vocab rows owned per core
BLOCKS = 98                # 12544 rows processed per core (128*98)
ROWS = BLOCKS * P
BF16 = ml_dtypes.bfloat16

CHUNKS = [2, 16, 16, 16, 16, 16, 16]   # input DMA chunk sizes (blocks);
# 16 blocks = exactly two super-tiles, so no tile waits on two chunk
# semaphores and completion sems pipeline at ~1.5us intervals
TILE_BLOCKS = 8                # blocks per compute super-tile
MMW = 512                      # single-matmul width (one PSUM fp32 bank)
LOOKAHEAD = 2                  # super-tiles in flight between mm1 and mm2


def build_nc():
    f32 = mybir.dt.float32
    bf16 = mybir.dt.bfloat16
    nc = bacc.Bacc("TRN2", target_bir_lowering=False, debug=False,
                   num_devices=N_CORES)

    tsl_t = nc.dram_tensor("tslice", [P, ROWS], bf16,
                           kind="ExternalInput").ap()
    wpack_t = nc.dram_tensor("wpack", [P, 2 * D], bf16,
                             kind="ExternalInput").ap()
    bpack_t = nc.dram_tensor("bpack", [P, 2], f32,
                             kind="ExternalInput").ap()
    out_t = nc.dram_tensor("out", [P, ROWS], bf16,
                           kind="ExternalOutput").ap()

    fw_max = TILE_BLOCKS * P  # 1024

    # super-tiles: one small starter tile, then 1024-column tiles
    tiles = []
    s0 = 0
    first = CHUNKS[0] * P
    tiles.append((s0, first))
    s0 += first
    while s0 < ROWS:
        fw = min(fw_max, ROWS - s0)
        tiles.append((s0, fw))
        s0 += fw

    # output groups (tile counts): big batches early for few descriptors,
    # a small final group so the last DMA after the last relu2 is short
    group_sizes = [4, 4, 4, 1]
    assert sum(group_sizes) == len(tiles)
    group_ends = []
    i = 0
    for g in group_sizes:
        i += g
        group_ends.append(tiles[i - 1][0] + tiles[i - 1][1])

    with FastExitTileContext(nc) as tc:
        with (
            tc.tile_pool(name="const", bufs=1) as cpool,
            tc.tile_pool(name="io", bufs=1) as iopool,
            tc.tile_pool(name="work", bufs=3) as wpool,
            tc.tile_pool(name="psum", bufs=2, space="PSUM") as ppool,
        ):
            # consts go on the Activation queue so the Sync queue's first
            # DIRECT2D is the first input chunk (ACT is idle until the
            # first relu, well after these land)
            wpack_sb = cpool.tile([P, 2 * D], bf16, tag="wpack")
            nc.scalar.dma_start(out=wpack_sb[:], in_=wpack_t[:])
            bpack_sb = cpool.tile([P, 2], f32, tag="bpack")
            nc.scalar.dma_start(out=bpack_sb[:], in_=bpack_t[:])
            w1t_sb = wpack_sb[:, 0:D]
            w2t_sb = wpack_sb[:, D:2 * D]
            b1_sb = bpack_sb[:, 0:1]
            b2_sb = bpack_sb[:, 1:2]

            # one big input tile; all chunk DMAs trigger up front
            win = iopool.tile([P, ROWS], bf16, tag="win")
            r = 0
            for cb in CHUNKS:
                cw = cb * P
                nc.sync.dma_start(out=win[:, r:r + cw],
                                  in_=tsl_t[:, r:r + cw])
                r += cw

            # output staging tile; relu2 writes slices, GpSimd drains
            ostage = iopool.tile([P, ROWS], bf16, tag="ostage")

            def mm_layer(out_ps, lhsT, rhs_ap, fw):
                for j in range(0, fw, MMW):
                    w = min(MMW, fw - j)
                    nc.tensor.matmul(out=out_ps[:, j:j + w], lhsT=lhsT,
                                     rhs=rhs_ap[:, j:j + w],
                                     start=True, stop=True)

            def stage_a(s0, fw):
                """mm1 + relu1(ACT) for one super-tile."""
                h_ps = ppool.tile([P, fw_max], f32, tag="h")
                mm_layer(h_ps, w1t_sb, win[:, s0:s0 + fw], fw)
                hT_sb = wpool.tile([P, fw_max], bf16, tag="hT")
                nc.scalar.activation(hT_sb[:, :fw], h_ps[:, :fw],
                                     mybir.ActivationFunctionType.Relu,
                                     bias=b1_sb)
                return hT_sb

            def stage_b(hT_sb, s0, fw):
                """mm2 + relu2(DVE) into the staging tile."""
                o_ps = ppool.tile([P, fw_max], f32, tag="o")
                mm_layer(o_ps, w2t_sb, hT_sb[:, :fw], fw)
                nc.vector.tensor_scalar(
                    out=ostage[:, s0:s0 + fw], in0=o_ps[:, :fw],
                    scalar1=b2_sb, scalar2=0.0,
                    op0=mybir.AluOpType.add, op1=mybir.AluOpType.max)

            pend = []  # (hT_sb, s0, fw) of in-flight super-tiles
            g_lo = 0
            gi = 0

            def maybe_drain(done_cols):
                nonlocal g_lo, gi
                if gi < len(group_ends) and done_cols >= group_ends[gi]:
                    hi = group_ends[gi]
                    nc.gpsimd.dma_start(out=out_t[:, g_lo:hi],
                                        in_=ostage[:, g_lo:hi])
                    g_lo = hi
                    gi += 1

            for (s0, fw) in tiles:
                hT_sb = stage_a(s0, fw)
                pend.append((hT_sb, s0, fw))
                if len(pend) > LOOKAHEAD:
                    b = pend.pop(0)
                    stage_b(*b)
                    maybe_drain(b[1] + b[2])
            for b in pend:
                stage_b(*b)
                maybe_drain(b[1] + b[2])

    nc.compile()
    return nc


_CACHED_NC = None
LAST_RESULTS = None


def _get_nc():
    global _CACHED_NC
    if _CACHED_NC is None:
        _CACHED_NC = build_nc()
    return _CACHED_NC


def kernel(nodes, c2e_weight, w1, b1, w2, b2):
    nodes = np.asarray(nodes).astype(np.int64)
    c2e_weight = np.asarray(c2e_weight, dtype=np.float32)
    w1 = np.asarray(w1, dtype=np.float32)
    b1 = np.asarray(b1, dtype=np.float32)
    w2 = np.asarray(w2, dtype=np.float32)
    b2 = np.asarray(b2, dtype=np.float32)

    vocab = c2e_weight.shape[0]
    assert vocab == VOCAB, vocab

    tableT = c2e_weight.T  # [128, VOCAB], d-major view

    wpack = np.concatenate(
        [np.ascontiguousarray(w1.T), np.ascontiguousarray(w2.T)],
        axis=1).astype(BF16)
    bpack = np.stack([b1, b2], axis=1).astype(np.float32)
    bpack = np.ascontiguousarray(bpack)

    starts = []
    in_maps = []
    for i in range(N_CORES):
        start = min(i * RANGE, vocab - ROWS)
        starts.append(start)
        in_maps.append({
            "tslice": np.ascontiguousarray(
                tableT[:, start:start + ROWS]).astype(BF16),
            "wpack": wpack,
            "bpack": bpack,
        })

    nc = _get_nc()
    trace = os.environ.get("BASS_KERNEL_TRACE") == "1"
    if trace:
        try:  # tracing needs the NTFF hook; degrade silently without it
            import antenv.axon_hooks  # noqa: F401
        except ImportError:
            trace = False
    res = run_bass_kernel_spmd(nc, in_maps, core_ids=list(range(N_CORES)),
                               trace=trace)
    global LAST_RESULTS
    LAST_RESULTS = res

    # T2[v] = MLP(c2e_weight[v]) assembled from the 8 windows
    t2 = np.empty((vocab, D), dtype=np.float32)
    for i in range(N_CORES):
        dense = res.results[i]["out"]                    # [128, ROWS] bf16
        lo = i * RANGE
        hi = min((i + 1) * RANGE, vocab)
        t2[lo:hi] = dense[:, lo - starts[i]:hi - starts[i]].T

    return t2[nodes]


# revision 15
# speedup vs baseline: 1.0611x; 1.0009x over previous
"""Trainium2 Bass kernel for the Context Encoder problem:

    ce  = c2e_weight[nodes]            # [N, 128] embedding gather
    h   = relu(ce @ w1.T + b1)         # [N, 128]
    out = relu(h @ w2.T + b2)          # [N, 128]

Strategy (8 NeuronCores, vocab-range sharding):
  200000 node ids over a 100000-row vocab saturate every vocab window,
  so transforming the table itself is less work than gathering per-node
  rows (and avoids the per-index DMA descriptor-generation cost that
  dominates any on-device gather).

  - The vocab is split into 8 fixed 12500-row ranges.  Core i streams
    its host-pre-transposed (d-major) bf16 table window [128, 12544]
    contiguously at full DMA bandwidth and computes
    T2 = relu(relu(win @ w1.T + b1) @ w2.T + b2) for every window row.
  - bf16 operands run the PE at 1 cycle/column (4x the fp32 rate) and
    halve both DMA directions; PSUM accumulation stays fp32 so the
    only precision loss is the bf16 quantization of table/weights/out
    (~4e-3 rel vs the 2e-2 gate).
  - All input-chunk DMAs are triggered up front on the Sync queue so
    the input stream never stalls behind compute; outputs accumulate
    in an SBUF staging tile and leave as four large DMAs issued from
    the otherwise-idle GpSimd queue.  Each dma_start costs ~600ns of
    sequencer descriptor generation, so DMA count is kept minimal.
  - Super-tiles span two PSUM banks (1024 columns); each layer is two
    512-column matmuls back-to-back, then a single 1024-wide bias+relu
    (layer 1 on ACT, layer 2 on DVE).  Tiles are software-pipelined one
    deep so mm2 of tile k-1 issues after mm1 of tile k and the PE does
    not stall on the ACT engine.
  - Results stay feature-major; the host transposes each window and maps
    node positions to rows (out = T2[nodes]) as the unshard step.
"""

import os
import sys

for _p in ("/opt/trn_rl_repo",):
    if _p not in sys.path:
        sys.path.insert(0, _p)

import ml_dtypes
import numpy as np

import concourse.bass as bass
import concourse.mybir as mybir
import concourse.tile as tile
from concourse import bacc
from concourse.bass_utils import run_bass_kernel_spmd
from concourse.tile import TileContext
from concourse.vector_clock import ScopedClock


class FastExitTileContext(TileContext):
    """Single-use TileContext with a cheap exit sequence.

    The stock exit runs sync-drain -> full barrier -> sem clear -> full
    barrier, where each full barrier emits a per-engine InstDrain whose
    release chain serializes at ~1.3us per engine on TRN2 (~8us of
    measured kernel time).  The sync drain already waits on every
    completion semaphore (including all DMA queues), so for a program
    that ends right after the TileContext a sem-only barrier before the
    clear is sufficient, and nothing needs to run after the clear.
    """

    def _drain_and_barrier(self, tick_clock, wait_clock):
        drain_inst = self.nc.sync.drain()
        wait_clock.add_sem_waits(
            drain_inst.ins, ScopedClock({None: tick_clock.global_clock})
        )
        popped = self.nc._tile_sem_poison_stack.pop()
        assert popped is self._sem_poison
        # No trailing barrier or semaphore clear: the program ends here
        # and the runtime re-initializes semaphore state on each NEFF
        # execution (verified by the double-run check in the harness).
        # The sync drain above still waits on every completion semaphore
        # including all DMA queues, so no engine stream can signal
        # completion while transfers are in flight.

P = 128
D = 128
N_CORES = 8
VOCAB = 100000
RANGE = VOCAB // N_CORES   # 12500 vocab rows owned per core
BLOCKS = 98                # 12544 rows processed per core (128*98)
ROWS = BLOCKS * P
BF16 = ml_dtypes.bfloat16

CHUNKS = [2, 16, 16, 16, 16, 16, 16]   # input DMA chunk sizes (blocks);
# 16 blocks = exactly two super-tiles, so no tile waits on two chunk
# semaphores and completion sems pipeline at ~1.5us intervals
TILE_BLOCKS = 8                # blocks per compute super-tile
MMW = 512                      # single-matmul width (one PSUM fp32 bank)
LOOKAHEAD = 2                  # super-tiles in flight between mm1 and mm2


def build_nc():
    f32 = mybir.dt.float32
    bf16 = mybir.dt.bfloat16
    nc = bacc.Bacc("TRN2", target_bir_lowering=False, debug=False,
                   num_devices=N_CORES)

    tsl_t = nc.dram_tensor("tslice", [P, ROWS], bf16,
                           kind="ExternalInput").ap()
    wpack_t = nc.dram_tensor("wpack", [P, 2 * D], bf16,
                             kind="ExternalInput").ap()
    bpack_t = nc.dram_tensor("bpack", [P, 2], f32,
                             kind="ExternalInput").ap()
    out_t = nc.dram_tensor("out", [P, ROWS], bf16,
                           kind="ExternalOutput").ap()

    fw_max = TILE_BLOCKS * P  # 1024

    # super-tiles: one small starter tile, then 1024-column tiles
    tiles = []
    s0 = 0
    first = CHUNKS[0] * P
    tiles.append((s0, first))
    s0 += first
    while s0 < ROWS:
        fw = min(fw_max, ROWS - s0)
        tiles.append((s0, fw))
        s0 += fw

    # output groups (tile counts): big batches early for few descriptors,
    # a small final group so the last DMA after the last relu2 is short
    group_sizes = [4, 4, 4, 1]
    assert sum(group_sizes) == len(tiles)
    group_ends = []
    i = 0
    for g in group_sizes:
        i += g
        group_ends.append(tiles[i - 1][0] + tiles[i - 1][1])

    with FastExitTileContext(nc) as tc:
        with (
            tc.tile_pool(name="const", bufs=1) as cpool,
            tc.tile_pool(name="io", bufs=1) as iopool,
            tc.tile_pool(name="work", bufs=3) as wpool,
            tc.tile_pool(name="psum", bufs=2, space="PSUM") as ppool,
        ):
            # consts go on the Activation queue so the Sync queue's first
            # DIRECT2D is the first input chunk (ACT is idle until the
            # first relu, well after these land)
            wpack_sb = cpool.tile([P, 2 * D], bf16, tag="wpack")
            nc.scalar.dma_start(out=wpack_sb[:], in_=wpack_t[:])
            bpack_sb = cpool.tile([P, 2], f32, tag="bpack")
            nc.scalar.dma_start(out=bpack_sb[:], in_=bpack_t[:])
            w1t_sb = wpack_sb[:, 0:D]
            w2t_sb = wpack_sb[:, D:2 * D]
            b1_sb = bpack_sb[:, 0:1]
            b2_sb = bpack_sb[:, 1:2]

            # one big input tile; all chunk DMAs trigger up front
            win = iopool.tile([P, ROWS], bf16, tag="win")
            r = 0
            for cb in CHUNKS:
                cw = cb * P
                nc.sync.dma_start(out=win[:, r:r + cw],
                                  in_=tsl_t[:, r:r + cw])
                r += cw

            # output staging tile; relu2 writes slices, GpSimd drains
            ostage = iopool.tile([P, ROWS], bf16, tag="ostage")

            def mm_layer(out_ps, lhsT, rhs_ap, fw):
                for j in range(0, fw, MMW):
                    w = min(MMW, fw - j)
                    nc.tensor.matmul(out=out_ps[:, j:j + w], lhsT=lhsT,
                                     rhs=rhs_ap[:, j:j + w],
                                     start=True, stop=True)

            def stage_a(s0, fw):
                """mm1 + relu1(ACT) for one super-tile."""
                h_ps = ppool.tile([P, fw_max], f32, tag="h")
                mm_layer(h_ps, w1t_sb, win[:, s0:s0 + fw], fw)
                hT_sb = wpool.tile([P, fw_max], bf16, tag="hT")
                nc.scalar.activation(hT_sb[:, :fw], h_ps[:, :fw],
                                     mybir.ActivationFunctionType.Relu,
                                     bias=b1_sb)
                return hT_sb

            def stage_b(hT_sb, s0, fw):
                """mm2 + relu2(DVE) into the staging tile."""
                o_ps = ppool.tile([P, fw_max], f32, tag="o")
                mm_layer(o_ps, w2t_sb, hT_sb[:, :fw], fw)
                nc.vector.tensor_scalar(
                    out=ostage[:, s0:s0 + fw], in0=o_ps[:, :fw],
                    scalar1=b2_sb, scalar2=0.0,
                    op0=mybir.AluOpType.add, op1=mybir.AluOpType.max)

            pend = []  # (hT_sb, s0, fw) of in-flight super-tiles
            g_lo = 0
            gi = 0

            def maybe_drain(done_cols):
                nonlocal g_lo, gi
                if gi < len(group_ends) and done_cols >= group_ends[gi]:
                    hi = group_ends[gi]
                    nc.gpsimd.dma_start(out=out_t[:, g_lo:hi],
                                        in_=ostage[:, g_lo:hi])
                    g_lo = hi
                    gi += 1

            for (s0, fw) in tiles:
                hT_sb = stage_a(s0, fw)
                pend.append((hT_sb, s0, fw))
                if len(pend) > LOOKAHEAD:
                    b = pend.pop(0)
                    stage_b(*b)
                    maybe_drain(b[1] + b[2])
            for b in pend:
                stage_b(*b)
                maybe_drain(b[1] + b[2])

    nc.compile()
    return nc


_CACHED_NC = None
LAST_RESULTS = None


def _get_nc():
    global _CACHED_NC
    if _CACHED_NC is None:
        _CACHED_NC = build_nc()
    return _CACHED_NC


def kernel(nodes, c2e_weight, w1, b1, w2, b2):
    nodes = np.asarray(nodes).astype(np.int64)
    c2e_weight = np.asarray(c2e_weight, dtype=np.float32)
    w1 = np.asarray(w1, dtype=np.float32)
    b1 = np.asarray(b1, dtype=np.float32)
    w2 = np.asarray(w2, dtype=np.float32)
    b2 = np.asarray(b2, dtype=np.float32)

    vocab = c2e_weight.shape[0]
    assert vocab == VOCAB, vocab

    tableT = c2e_weight.T  # [128, VOCAB], d-major view

    wpack = np.concatenate(
        [np.ascontiguousarray(w1.T), np.ascontiguousarray(w2.T)],
        axis=1).astype(BF16)
    bpack = np.stack([b1, b2], axis=1).astype(np.float32)
    bpack = np.ascontiguousarray(bpack)

    starts = []
    in_maps = []
    for i in range(N_CORES):
        start = min(i * RANGE, vocab - ROWS)
        starts.append(start)
        in_maps.append({
            "tslice": np.ascontiguousarray(
                tableT[:, start:start + ROWS]).astype(BF16),
            "wpack": wpack,
            "bpack": bpack,
        })

    nc = _get_nc()
    trace = os.environ.get("BASS_KERNEL_TRACE") == "1"
    if trace:
        try:  # tracing needs the NTFF hook; degrade silently without it
            import antenv.axon_hooks  # noqa: F401
        except ImportError:
            trace = False
    res = run_bass_kernel_spmd(nc, in_maps, core_ids=list(range(N_CORES)),
                               trace=trace)
    global LAST_RESULTS
    LAST_RESULTS = res

    # T2[v] = MLP(c2e_weight[v]) assembled from the 8 windows
    t2 = np.empty((vocab, D), dtype=np.float32)
    for i in range(N_CORES):
        dense = res.results[i]["out"]                    # [128, ROWS] bf16
        lo = i * RANGE
        hi = min((i + 1) * RANGE, vocab)
        t2[lo:hi] = dense[:, lo - starts[i]:hi - starts[i]].T

    return t2[nodes]


# revision 23
# speedup vs baseline: 1.0724x; 1.0106x over previous
"""Trainium2 Bass kernel for the Context Encoder problem:

    ce  = c2e_weight[nodes]            # [N, 128] embedding gather
    h   = relu(ce @ w1.T + b1)         # [N, 128]
    out = relu(h @ w2.T + b2)          # [N, 128]

Strategy (8 NeuronCores, vocab-range sharding):
  200000 node ids over a 100000-row vocab saturate every vocab window,
  so transforming the table itself is less work than gathering per-node
  rows (and avoids the per-index DMA descriptor-generation cost that
  dominates any on-device gather).

  - The vocab is split into 8 fixed 12500-row ranges.  Core i streams
    its host-pre-transposed (d-major) bf16 table window [128, 12544]
    contiguously at full DMA bandwidth and computes
    T2 = relu(relu(win @ w1.T + b1) @ w2.T + b2) for every window row.
  - bf16 operands run the PE at 1 cycle/column (4x the fp32 rate) and
    halve both DMA directions; PSUM accumulation stays fp32 so the
    only precision loss is the bf16 quantization of table/weights/out
    (~4e-3 rel vs the 2e-2 gate).
  - All input-chunk DMAs are triggered up front on the Sync queue so
    the input stream never stalls behind compute; outputs accumulate
    in an SBUF staging tile and leave as four large DMAs issued from
    the otherwise-idle GpSimd queue.  Each dma_start costs ~600ns of
    sequencer descriptor generation, so DMA count is kept minimal.
  - Super-tiles span two PSUM banks (1024 columns); each layer is two
    512-column matmuls back-to-back, then a single 1024-wide bias+relu
    (layer 1 on ACT, layer 2 on DVE).  Tiles are software-pipelined one
    deep so mm2 of tile k-1 issues after mm1 of tile k and the PE does
    not stall on the ACT engine.
  - Results stay feature-major; the host transposes each window and maps
    node positions to rows (out = T2[nodes]) as the unshard step.
"""

import os
import sys

for _p in ("/opt/trn_rl_repo",):
    if _p not in sys.path:
        sys.path.insert(0, _p)

import ml_dtypes
import numpy as np

import concourse.bass as bass
import concourse.mybir as mybir
import concourse.tile as tile
from concourse import bacc
from concourse.bass_utils import run_bass_kernel_spmd
from concourse.tile import TileContext
from concourse.vector_clock import ScopedClock


class FastExitTileContext(TileContext):
    """Single-use TileContext with a cheap exit sequence.

    The stock exit runs sync-drain -> full barrier -> sem clear -> full
    barrier, where each full barrier emits a per-engine InstDrain whose
    release chain serializes at ~1.3us per engine on TRN2 (~8us of
    measured kernel time).  The sync drain already waits on every
    completion semaphore (including all DMA queues), so for a program
    that ends right after the TileContext a sem-only barrier before the
    clear is sufficient, and nothing needs to run after the clear.
    """

    def _drain_and_barrier(self, tick_clock, wait_clock):
        drain_inst = self.nc.sync.drain()
        wait_clock.add_sem_waits(
            drain_inst.ins, ScopedClock({None: tick_clock.global_clock})
        )
        popped = self.nc._tile_sem_poison_stack.pop()
        assert popped is self._sem_poison
        # No trailing barrier or semaphore clear: the program ends here
        # and the runtime re-initializes semaphore state on each NEFF
        # execution (verified by the double-run check in the harness).
        # The sync drain above still waits on every completion semaphore
        # including all DMA queues, so no engine stream can signal
        # completion while transfers are in flight.

P = 128
D = 128
N_CORES = 8
VOCAB = 100000
RANGE = VOCAB // N_CORES   # 12500 vocab rows owned per core
BLOCKS = 98                # 12544 rows processed per core (128*98)
ROWS = BLOCKS * P
BF16 = ml_dtypes.bfloat16

# input DMA chunk sizes (blocks); 16 blocks = exactly two super-tiles,
# so no tile waits on two chunk semaphores and completion sems (~2us
# posting lag each) pipeline at ~1.5us intervals
CHUNKS = [2, 16, 16, 16, 16, 16, 16]
TILE_SIZES = [2] + [8] * 12            # super-tile sizes (blocks)
TILE_BLOCKS = 8                # blocks per compute super-tile
MMW = 512                      # single-matmul width (one PSUM fp32 bank)
LOOKAHEAD = 2                  # super-tiles in flight between mm1 and mm2


def build_nc():
    f32 = mybir.dt.float32
    bf16 = mybir.dt.bfloat16
    nc = bacc.Bacc("TRN2", target_bir_lowering=False, debug=False,
                   num_devices=N_CORES)

    tsl_t = nc.dram_tensor("tslice", [P, ROWS], bf16,
                           kind="ExternalInput").ap()
    wpack_t = nc.dram_tensor("wpack", [P, 2 * D], bf16,
                             kind="ExternalInput").ap()
    bpack_t = nc.dram_tensor("bpack", [P, 2], f32,
                             kind="ExternalInput").ap()
    out_t = nc.dram_tensor("out", [P, ROWS], bf16,
                           kind="ExternalOutput").ap()

    fw_max = TILE_BLOCKS * P  # 1024

    assert sum(CHUNKS) == BLOCKS and sum(TILE_SIZES) == BLOCKS
    tiles = []
    s0 = 0
    for tb in TILE_SIZES:
        tiles.append((s0, tb * P))
        s0 += tb * P

    # output groups (tile counts): big batches early for few descriptors,
    # a small final group so the last DMA after the last relu2 is short
    group_sizes = [4, 4, 4, 1]
    assert sum(group_sizes) == len(tiles)
    group_ends = []
    i = 0
    for g in group_sizes:
        i += g
        group_ends.append(tiles[i - 1][0] + tiles[i - 1][1])

    with FastExitTileContext(nc) as tc:
        with (
            tc.tile_pool(name="const", bufs=1) as cpool,
            tc.tile_pool(name="io", bufs=1) as iopool,
            tc.tile_pool(name="work", bufs=3) as wpool,
            tc.tile_pool(name="psum", bufs=2, space="PSUM") as ppool,
        ):
            # consts go on the Activation queue so the Sync queue's first
            # DIRECT2D is the first input chunk (ACT is idle until the
            # first relu, well after these land)
            wpack_sb = cpool.tile([P, 2 * D], bf16, tag="wpack")
            nc.scalar.dma_start(out=wpack_sb[:], in_=wpack_t[:])
            bpack_sb = cpool.tile([P, 2], f32, tag="bpack")
            nc.scalar.dma_start(out=bpack_sb[:], in_=bpack_t[:])
            w1t_sb = wpack_sb[:, 0:D]
            w2t_sb = wpack_sb[:, D:2 * D]
            b1_sb = bpack_sb[:, 0:1]
            b2_sb = bpack_sb[:, 1:2]

            # one big input tile; all chunk DMAs trigger up front
            win = iopool.tile([P, ROWS], bf16, tag="win")
            r = 0
            for cb in CHUNKS:
                cw = cb * P
                nc.sync.dma_start(out=win[:, r:r + cw],
                                  in_=tsl_t[:, r:r + cw])
                r += cw

            # output staging tile; relu2 writes slices, GpSimd drains
            ostage = iopool.tile([P, ROWS], bf16, tag="ostage")

            def mm_layer(out_ps, lhsT, rhs_ap, fw):
                for j in range(0, fw, MMW):
                    w = min(MMW, fw - j)
                    nc.tensor.matmul(out=out_ps[:, j:j + w], lhsT=lhsT,
                                     rhs=rhs_ap[:, j:j + w],
                                     start=True, stop=True)

            def stage_a(s0, fw):
                """mm1 + relu1(ACT) for one super-tile."""
                h_ps = ppool.tile([P, fw_max], f32, tag="h")
                mm_layer(h_ps, w1t_sb, win[:, s0:s0 + fw], fw)
                hT_sb = wpool.tile([P, fw_max], bf16, tag="hT")
                nc.scalar.activation(hT_sb[:, :fw], h_ps[:, :fw],
                                     mybir.ActivationFunctionType.Relu,
                                     bias=b1_sb)
                return hT_sb

            def stage_b(hT_sb, s0, fw):
                """mm2 + relu2(DVE) into the staging tile."""
                o_ps = ppool.tile([P, fw_max], f32, tag="o")
                mm_layer(o_ps, w2t_sb, hT_sb[:, :fw], fw)
                nc.vector.tensor_scalar(
                    out=ostage[:, s0:s0 + fw], in0=o_ps[:, :fw],
                    scalar1=b2_sb, scalar2=0.0,
                    op0=mybir.AluOpType.add, op1=mybir.AluOpType.max)

            pend = []  # (hT_sb, s0, fw) of in-flight super-tiles
            g_lo = 0
            gi = 0

            def maybe_drain(done_cols):
                nonlocal g_lo, gi
                if gi < len(group_ends) and done_cols >= group_ends[gi]:
                    hi = group_ends[gi]
                    nc.gpsimd.dma_start(out=out_t[:, g_lo:hi],
                                        in_=ostage[:, g_lo:hi])
                    g_lo = hi
                    gi += 1

            for (s0, fw) in tiles:
                hT_sb = stage_a(s0, fw)
                pend.append((hT_sb, s0, fw))
                if len(pend) > LOOKAHEAD:
                    b = pend.pop(0)
                    stage_b(*b)
                    maybe_drain(b[1] + b[2])
            for b in pend:
                stage_b(*b)
                maybe_drain(b[1] + b[2])

    nc.compile()
    return nc


_CACHED_NC = None
LAST_RESULTS = None


def _get_nc():
    global _CACHED_NC
    if _CACHED_NC is None:
        _CACHED_NC = build_nc()
    return _CACHED_NC


def kernel(nodes, c2e_weight, w1, b1, w2, b2):
    nodes = np.asarray(nodes).astype(np.int64)
    c2e_weight = np.asarray(c2e_weight, dtype=np.float32)
    w1 = np.asarray(w1, dtype=np.float32)
    b1 = np.asarray(b1, dtype=np.float32)
    w2 = np.asarray(w2, dtype=np.float32)
    b2 = np.asarray(b2, dtype=np.float32)

    vocab = c2e_weight.shape[0]
    assert vocab == VOCAB, vocab

    tableT = c2e_weight.T  # [128, VOCAB], d-major view

    wpack = np.concatenate(
        [np.ascontiguousarray(w1.T), np.ascontiguousarray(w2.T)],
        axis=1).astype(BF16)
    bpack = np.stack([b1, b2], axis=1).astype(np.float32)
    bpack = np.ascontiguousarray(bpack)

    starts = []
    in_maps = []
    for i in range(N_CORES):
        start = min(i * RANGE, vocab - ROWS)
        starts.append(start)
        in_maps.append({
            "tslice": np.ascontiguousarray(
                tableT[:, start:start + ROWS]).astype(BF16),
            "wpack": wpack,
            "bpack": bpack,
        })

    nc = _get_nc()
    trace = os.environ.get("BASS_KERNEL_TRACE") == "1"
    if trace:
        try:  # tracing needs the NTFF hook; degrade silently without it
            import antenv.axon_hooks  # noqa: F401
        except ImportError:
            trace = False
    res = run_bass_kernel_spmd(nc, in_maps, core_ids=list(range(N_CORES)),
                               trace=trace)
    global LAST_RESULTS
    LAST_RESULTS = res

    # T2[v] = MLP(c2e_weight[v]) assembled from the 8 windows
    t2 = np.empty((vocab, D), dtype=np.float32)
    for i in range(N_CORES):
        dense = res.results[i]["out"]                    # [128, ROWS] bf16
        lo = i * RANGE
        hi = min((i + 1) * RANGE, vocab)
        t2[lo:hi] = dense[:, lo - starts[i]:hi - starts[i]].T

    return t2[nodes]
